# revision 32
# baseline (speedup 1.0000x reference)
"""GCNII encoder + KNN label-fusion subgraph on 8 Trainium2 NeuronCores.

Sharding: nodes (rows) split into 8 blocks of N/8. Each core:
  - builds its dense fp16 adjacency block A^T[:, blk] ON DEVICE from a
    compact padded COO edge list (iota-compare one-hots + PE matmuls),
    so only ~6 MiB/core of inputs cross the host link instead of 512 MiB
  - computes h = relu(x_blk @ W_in + b_in)
  - 9 GCNII layers: agg_blk = A[blk, :] @ h_full (dense fp16 adjacency
    streamed from device HBM as PE matmuls), h_full re-AllGathered (fp16)
  - p_lc = log_softmax(emb @ W_out + b_out) on its rows
  - cosine-sim branch: en = emb/||emb||; per-row exact top-16 threshold tau
    via max8/match_replace8 over PSUM sim strips; fused = (exp(sim) *
    (sim >= tau)) @ one_hot(y) as PE matmuls; p_sim = log_softmax(fused)
  - out = 0.5*p_lc + 0.5*p_sim
Host only preps layouts: bucketed edge lists, transposed x, weights.
"""
import math
from contextlib import ExitStack

import numpy as np

import concourse.bass as bass
import concourse.tile as tile
from concourse import bacc, mybir
from concourse.bass_utils import run_bass_kernel_spmd
from concourse.masks import make_identity

F32 = mybir.dt.float32
F16 = mybir.dt.float16
I32 = mybir.dt.int32
AF = mybir.ActivationFunctionType
ALU = mybir.AluOpType

N_CORES = 8
N = 16384
D_IN = 512
H = 256
C = 64
K_TOP = 16
N_LAYERS = 9
ALPHA = 0.5
THETA = 1.0
NEG = -1e30

# edge bucketing: bucket = (src slab js, 256-wide dst window a), one 128-edge
# chunk per bucket. Mean fill 64, CAP 128 = +8 sigma; overflow ~impossible.
AW = 256             # dst window width per bucket
CAP = 128            # bucket capacity (one matmul chunk)
GW = 16              # A-build: js slabs batched per at_dram write DMA
GR = 8               # agg: js slabs batched per at_dram read DMA


def _betas():
    return [float(np.log(THETA / (l + 1) + 1.0)) for l in range(N_LAYERS)]


def build_program(n=N, n_layers=N_LAYERS, skip_abuild=False, skip_sim=False,
                  fake_collectives=False, abuild_mode='vec'):
    blk = n // N_CORES          # rows per core
    n_it = blk // 128           # 128-row tiles per block
    igw = min(512, blk)         # i-group width (dst cols per psum tile)
    n_ig = blk // igw
    n_js = n // 128             # src slabs
    chunkw = min(1024, n)       # S1 scan chunk width
    n_chunk = n // chunkw
    subw = min(512, blk)        # sim rhs tile width (<= c-block, <= 512)
    betas = _betas()

    nc = bacc.Bacc("TRN2", target_bir_lowering=False, debug=False,
                   num_devices=N_CORES)

    n_aw = blk // AW            # dst windows per core
    xT_d = nc.dram_tensor("xT16", [128, D_IN // 128, blk], F16,
                          kind="ExternalInput")
    esrc_d = nc.dram_tensor("esrc", [128, n_js, n_aw], F16,
                            kind="ExternalInput")
    edst_d = nc.dram_tensor("edst", [128, n_js, n_aw], F16,
                            kind="ExternalInput")
    ew_d = nc.dram_tensor("ew", [128, n_js, n_aw], F16,
                          kind="ExternalInput")
    y_d = nc.dram_tensor("y_r", [128, n_js], F16, kind="ExternalInput")
    w_in_d = nc.dram_tensor("w_in16", [128, D_IN // 128, H], F16,
                            kind="ExternalInput")
    b_in_d = nc.dram_tensor("b_in16", [1, H], F16, kind="ExternalInput")
    cw1_d = nc.dram_tensor("cw116", [N_LAYERS, 128, 2, H], F16,
                           kind="ExternalInput")
    cw2_d = nc.dram_tensor("cw216", [N_LAYERS, 128, 2, H], F16,
                           kind="ExternalInput")
    w_out_d = nc.dram_tensor("w_out_r", [128, 2, C], F32, kind="ExternalInput")
    b_out_d = nc.dram_tensor("b_out_r", [1, C], F32, kind="ExternalInput")
    out_d = nc.dram_tensor("out", [blk, C], F32, kind="ExternalOutput")

    groups = [list(range(N_CORES))]

    with tile.TileContext(nc) as tc, ExitStack() as S:
        const = S.enter_context(tc.tile_pool(name="const", bufs=1))
        dram = S.enter_context(tc.tile_pool(name="dram", bufs=1, space="DRAM"))
        hT_pool = S.enter_context(tc.tile_pool(name="hTp", bufs=2))
        # GCN-phase pools, released before the similarity phase
        G = ExitStack()
        x0pool = G.enter_context(tc.tile_pool(name="x0p", bufs=1))
        hfull_pool = G.enter_context(tc.tile_pool(name="hfp", bufs=1))
        h16b_pool = G.enter_context(tc.tile_pool(name="h16bp", bufs=2))

        ident = const.tile([128, 128], F32)
        make_identity(nc, ident[:])
        ident16 = const.tile([128, 128], F16)
        nc.vector.tensor_copy(ident16[:], ident[:])
        ones1 = const.tile([1, 128], F32)
        nc.vector.memset(ones1[:], 1.0)
        ones1_16 = const.tile([1, 128], F16)
        nc.vector.memset(ones1_16[:], 1.0)
        # iota tiles for one-hot construction (values exact in f16)
        iota_w_i = const.tile([128, igw], I32)
        nc.gpsimd.iota(iota_w_i[:], pattern=[[1, igw]], channel_multiplier=0)
        iota_w16 = const.tile([128, igw], F16)
        nc.vector.tensor_copy(iota_w16[:], iota_w_i[:])
        iota_p_i = const.tile([128, 128], I32)
        nc.gpsimd.iota(iota_p_i[:], pattern=[[1, 128]], channel_multiplier=0)
        iota_p16 = const.tile([128, 128], F16)
        nc.vector.tensor_copy(iota_p16[:], iota_p_i[:])
        iota_c = const.tile([128, C], F32)
        nc.vector.tensor_copy(iota_c[:], iota_p_i[:, :C])

        w_in_sb = const.tile([128, D_IN // 128, H], F16)
        nc.sync.dma_start(w_in_sb[:], w_in_d.ap())
        b_in_sb = const.tile([1, H], F16)
        nc.sync.dma_start(b_in_sb[:], b_in_d.ap())
        w_out_sb = const.tile([128, 2, C], F32)
        nc.sync.dma_start(w_out_sb[:], w_out_d.ap())
        b_out_sb = const.tile([1, C], F32)
        nc.sync.dma_start(b_out_sb[:], b_out_d.ap())

        # ablation-only: keep every NEFF input bound when a phase is skipped
        if skip_abuild:
            dmy = const.tile([128, n_js, n_aw], F16)
            nc.sync.dma_start(dmy[:], esrc_d.ap())
            dmy2 = const.tile([128, n_js, n_aw], F16)
            nc.sync.dma_start(dmy2[:], edst_d.ap())
            dmy3 = const.tile([128, n_js, n_aw], F16)
            nc.sync.dma_start(dmy3[:], ew_d.ap())
        if n_layers == 0:
            dmy4 = const.tile([128, 2, H], F16)
            nc.sync.dma_start(dmy4[:], cw1_d.ap()[0])
            dmy5 = const.tile([128, 2, H], F16)
            nc.sync.dma_start(dmy5[:], cw2_d.ap()[0])

        # ---------- y labels (one-hot built later, in the sim phase) ----------
        y16_sb = const.tile([128, n_js], F16)
        nc.sync.dma_start(y16_sb[:], y_d.ap())
        y_sb = const.tile([128, n_js], F32)
        nc.vector.tensor_copy(y_sb[:], y16_sb[:])

        x0sT = x0pool.tile([128, 2, blk], F32)
        x0sT16 = x0pool.tile([128, 2, blk], F16)
        out_acc = const.tile([128, n_it, C], F32)

        def logsoftmax_from_psum(dst_ap, psrc, sp, add_into=None):
            """dst = 0.5 * log_softmax(psrc rows); psrc is [128, C] psum."""
            m8 = sp.tile([128, 8], F32, tag="ls_m")
            nc.vector.max(out=m8[:], in_=psrc[:])
            m = m8[:, 0:1]
            mneg = sp.tile([128, 1], F32, tag="ls_mn")
            nc.vector.tensor_scalar_mul(mneg[:], m, -1.0)
            e = sp.tile([128, C], F32, tag="ls_e")
            ssum = sp.tile([128, 1], F32, tag="ls_s")
            nc.scalar.activation(e[:], psrc[:], AF.Exp, bias=mneg[:], scale=1.0,
                                 accum_out=ssum[:])
            ls = sp.tile([128, 1], F32, tag="ls_l")
            nc.scalar.activation(ls[:], ssum[:], AF.Ln)
            m2 = sp.tile([128, 1], F32, tag="ls_m2")
            nc.vector.tensor_add(m2[:], m, ls[:])
            if add_into is None:
                nc.vector.tensor_scalar(dst_ap, psrc[:], m2[:], 0.5,
                                        op0=ALU.subtract, op1=ALU.mult)
            else:
                t = sp.tile([128, C], F32, tag="ls_t")
                nc.vector.tensor_scalar(t[:], psrc[:], m2[:], 0.5,
                                        op0=ALU.subtract, op1=ALU.mult)
                nc.vector.tensor_add(dst_ap, add_into, t[:])

        def allgather_h16(h16_blk_t, tag):
            gin = dram.tile([128, n_it, H], F16, tag=f"{tag}_in")
            nc.sync.dma_start(gin[:], h16_blk_t[:])
            if fake_collectives:
                gout = dram.tile([N_CORES, 128, n_it, H], F16,
                                 tag=f"{tag}_out")
                for cc in range(N_CORES):
                    nc.sync.dma_start(gout[cc], gin[:])
            else:
                gout = dram.tile([N_CORES, 128, n_it, H], F16,
                                 tag=f"{tag}_out", addr_space="Shared")
                nc.gpsimd.collective_compute(
                    "AllGather", ALU.bypass, replica_groups=groups,
                    ins=[gin[:].opt()], outs=[gout[:].opt()])
            # one SBUF tile per core so agg matmuls on core jc's slabs only
            # wait for that core's copy, not the whole 8 MiB staging
            hf = []
            for cc in range(N_CORES):
                hf_c = hfull_pool.tile([128, n_it, H], F16, tag=f"hfull{cc}",
                                       name=f"hfull{cc}")
                nc.sync.dma_start(hf_c[:], gout[cc])
                hf.append(hf_c)
            return hf

        # ---------- phase 0: h0 = relu(x @ W_in + b_in) ----------
        with (
            tc.tile_pool(name="p0ps", bufs=2, space="PSUM") as p0ps,
            tc.tile_pool(name="p0sb", bufs=3) as p0sb,
            tc.tile_pool(name="p0x", bufs=1) as p0x,
        ):
            xT_sb = p0x.tile([128, D_IN // 128, blk], F16)
            nc.sync.dma_start(xT_sb[:], xT_d.ap())
            hT = hT_pool.tile([128, 2, blk], F32, tag="hT")
            h16_blk = h16b_pool.tile([128, n_it, H], F16, tag="h16b")
            for it in range(n_it):
                ph = p0ps.tile([128, H], F32, tag="ph")
                for k in range(D_IN // 128):
                    nc.tensor.matmul(ph[:], xT_sb[:, k, it * 128:(it + 1) * 128],
                                     w_in_sb[:, k, :], start=(k == 0), stop=False)
                nc.tensor.matmul(ph[:], ones1_16[:], b_in_sb[:], start=False,
                                 stop=True)
                hm = p0sb.tile([128, H], F32, tag="hm")
                nc.scalar.activation(hm[:], ph[:], AF.Relu)
                nc.vector.tensor_copy(h16_blk[:, it, :], hm[:])
                for dh in range(2):
                    pt = p0ps.tile([128, 128], F32, tag="pt")
                    nc.tensor.transpose(pt[:], hm[:, dh * 128:(dh + 1) * 128], ident[:])
                    nc.scalar.activation(hT[:, dh, it * 128:(it + 1) * 128], pt[:], AF.Copy)
            nc.vector.tensor_scalar_mul(x0sT[:], hT[:], 0.5)
            nc.vector.tensor_copy(x0sT16[:], x0sT[:])
        h16_full = allgather_h16(h16_blk, "ag")

        # ---------- build dense A^T on device from bucketed COO ----------
        # at_dram[p, ig, js, j] = sum over edges (src=js*128+p -> dst ig*igw+j)
        # partition-major so reads/writes batch GW/GR slabs per DMA with one
        # contiguous descriptor per partition.
        at_dram = dram.tile([128, n_ig, n_js, igw], F16)
        if not skip_abuild:
          with (
            tc.tile_pool(name="abps", bufs=4, space="PSUM") as abps,
            tc.tile_pool(name="absb", bufs=3) as absb,
            tc.tile_pool(name="abwr", bufs=2) as abwr,
            tc.tile_pool(name="aesb", bufs=1) as aesb,
          ):
            esrc16 = aesb.tile([128, n_js, n_aw], F16)
            nc.sync.dma_start(esrc16[:], esrc_d.ap())
            edst16 = aesb.tile([128, n_js, n_aw], F16)
            nc.sync.dma_start(edst16[:], edst_d.ap())
            ew16_t = aesb.tile([128, n_js, n_aw], F16)
            nc.sync.dma_start(ew16_t[:], ew_d.ap())
            esrc_sb = aesb.tile([128, n_js, n_aw], F32)
            nc.vector.tensor_copy(esrc_sb[:], esrc16[:])
            edst_sb = aesb.tile([128, n_js, n_aw], F32)
            nc.vector.tensor_copy(edst_sb[:], edst16[:])
            ew_sb = aesb.tile([128, n_js, n_aw], F32)
            nc.vector.tensor_copy(ew_sb[:], ew16_t[:])
            npw = igw // AW  # buckets per igw stripe
            for ig in range(n_ig):
                for jg in range(n_js // GW):
                    wr = abwr.tile([128, GW, igw], F16, tag="wr")
                    for jl in range(GW):
                        js = jg * GW + jl
                        for a2 in range(npw):
                            a = ig * npw + a2
                            pa = abps.tile([128, AW], F32, tag="pa")
                            ohs = absb.tile([128, 128], F16, tag="ohs")
                            nc.vector.tensor_scalar(
                                ohs[:], iota_p16[:], esrc_sb[:, js, a:a + 1],
                                None, op0=ALU.is_equal)
                            ohd = absb.tile([128, AW], F16, tag="ohd")
                            nc.vector.tensor_scalar(
                                ohd[:], iota_w16[:, :AW], edst_sb[:, js, a:a + 1],
                                ew_sb[:, js, a:a + 1],
                                op0=ALU.is_equal, op1=ALU.mult)
                            nc.tensor.matmul(pa[:], ohs[:], ohd[:],
                                             start=True, stop=True)
                            # alternate copy engine: ACT and DVE share the load
                            if (jl + a2) % 2 == 0:
                                nc.scalar.activation(
                                    wr[:, jl, a2 * AW:(a2 + 1) * AW], pa[:], AF.Copy)
                            else:
                                nc.vector.tensor_copy(
                                    wr[:, jl, a2 * AW:(a2 + 1) * AW], pa[:])
                    nc.sync.dma_start(at_dram[:, ig, jg * GW:(jg + 1) * GW, :],
                                      wr[:])

        # ---------- GCN layers ----------
        with (
            tc.tile_pool(name="aggps", bufs=2, space="PSUM") as aggps,
            tc.tile_pool(name="mmps", bufs=2, space="PSUM") as mmps,
            tc.tile_pool(name="tps", bufs=2, space="PSUM") as tps,
            tc.tile_pool(name="apool", bufs=3) as apool,
            tc.tile_pool(name="wpool", bufs=2) as wpool,
            tc.tile_pool(name="xpool", bufs=2) as xpool,
            tc.tile_pool(name="tpool", bufs=3) as tpool,
        ):
            for l in range(n_layers):
                beta = betas[l]
                cw1_sb = wpool.tile([128, 2, H], F16, tag="cw1")
                nc.sync.dma_start(cw1_sb[:], cw1_d.ap()[l])
                cw2_sb = wpool.tile([128, 2, H], F16, tag="cw2")
                nc.sync.dma_start(cw2_sb[:], cw2_d.ap()[l])
                hT_new = hT_pool.tile([128, 2, blk], F32, tag="hT")
                for ig in range(n_ig):
                    pa0 = aggps.tile([128, igw], F32, tag="agg0")
                    pa1 = aggps.tile([128, igw], F32, tag="agg1")
                    for g in range(n_js // GR):
                        a_t = apool.tile([128, GR, igw], F16, tag="a")
                        nc.sync.dma_start(a_t[:],
                                          at_dram[:, ig, g * GR:(g + 1) * GR, :])
                        for r in range(GR):
                            js = g * GR + r
                            jc, jb = divmod(js, n_it)
                            nc.tensor.matmul(pa0[:], h16_full[jc][:, jb, 0:128],
                                             a_t[:, r, :], start=(js == 0),
                                             stop=(js == n_js - 1))
                            nc.tensor.matmul(pa1[:], h16_full[jc][:, jb, 128:256],
                                             a_t[:, r, :], start=(js == 0),
                                             stop=(js == n_js - 1))
                    xsT = xpool.tile([128, 2, igw], F32, tag="xsT")
                    nc.scalar.activation(xsT[:, 0, :], pa0[:], AF.Copy, scale=0.5)
                    nc.scalar.activation(xsT[:, 1, :], pa1[:], AF.Copy, scale=0.5)
                    xsT16 = xpool.tile([128, 2, igw], F16, tag="xsT16")
                    nc.vector.tensor_copy(xsT16[:], xsT[:])
                    sl = slice(ig * igw, (ig + 1) * igw)
                    for dh in range(2):
                        # cw2 terms first: they only need x0sT16, so the PE can
                        # start them while the AllGather/agg is still in flight
                        pmm = mmps.tile([128, igw], F32, tag="pmm")
                        nc.tensor.matmul(pmm[:], cw2_sb[:, 0, dh * 128:(dh + 1) * 128],
                                         x0sT16[:, 0, sl], start=True, stop=False)
                        nc.tensor.matmul(pmm[:], cw2_sb[:, 1, dh * 128:(dh + 1) * 128],
                                         x0sT16[:, 1, sl], start=False, stop=False)
                        nc.tensor.matmul(pmm[:], cw1_sb[:, 0, dh * 128:(dh + 1) * 128],
                                         xsT16[:, 0, :], start=False, stop=False)
                        nc.tensor.matmul(pmm[:], cw1_sb[:, 1, dh * 128:(dh + 1) * 128],
                                         xsT16[:, 1, :], start=False, stop=True)
                        t1 = tpool.tile([128, igw], F32, tag="t1")
                        nc.vector.tensor_add(t1[:], xsT[:, dh, :], x0sT[:, dh, sl])
                        t2 = tpool.tile([128, igw], F32, tag="t2")
                        nc.scalar.activation(t2[:], pmm[:], AF.Copy, scale=beta)
                        nc.vector.tensor_scalar_mul(t1[:], t1[:], 1.0 - beta)
                        nc.vector.tensor_add(t1[:], t1[:], t2[:])
                        nc.vector.tensor_add(t1[:], t1[:], hT[:, dh, sl])
                        nc.scalar.activation(hT_new[:, dh, sl], t1[:], AF.Relu)
                hT = hT_new
                if l < n_layers - 1:
                    h16_new = h16b_pool.tile([128, n_it, H], F16, tag="h16b")
                    for it in range(n_it):
                        for dh in range(2):
                            pt = tps.tile([128, 128], F32, tag="pt")
                            nc.tensor.transpose(
                                pt[:], hT[:, dh, it * 128:(it + 1) * 128], ident[:])
                            nc.scalar.activation(
                                h16_new[:, it, dh * 128:(dh + 1) * 128], pt[:], AF.Copy)
                    h16_full = allgather_h16(h16_new, "ag")
        embT = hT  # [128, 2, blk] f32
        G.close()  # release GCN-phase SBUF (h16_full, x0sT, h16_blk)
        spool = S.enter_context(tc.tile_pool(name="spool", bufs=1))

        # ---------- p_lc ----------
        with (
            tc.tile_pool(name="lcps", bufs=2, space="PSUM") as lcps,
            tc.tile_pool(name="lcsb", bufs=2) as lcsb,
        ):
            for it in range(n_it):
                plc = lcps.tile([128, C], F32, tag="plc")
                nc.tensor.matmul(plc[:], embT[:, 0, it * 128:(it + 1) * 128],
                                 w_out_sb[:, 0, :], start=True, stop=False)
                nc.tensor.matmul(plc[:], embT[:, 1, it * 128:(it + 1) * 128],
                                 w_out_sb[:, 1, :], start=False, stop=False)
                nc.tensor.matmul(plc[:], ones1[:], b_out_sb[:], start=False, stop=True)
                logsoftmax_from_psum(out_acc[:, it, :], plc, lcsb)

        if not skip_sim:
          # ---------- one-hot(y) (deferred from init to keep SBUF free) ----------
          oh_sb = spool.tile([128, n_js, C], F16)
          for js in range(n_js):
              nc.vector.tensor_scalar(oh_sb[:, js, :], iota_c[:],
                                      y_sb[:, js:js + 1], None, op0=ALU.is_equal)
          # ---------- normalize ----------
          enT16_blk = spool.tile([128, 2, blk], F16)
          with (
              tc.tile_pool(name="nps", bufs=2, space="PSUM") as nps,
              tc.tile_pool(name="nsb", bufs=3) as nsb,
          ):
              en16_blk = nsb.tile([128, n_it, H], F16, tag="en16b", bufs=1)
              eps_t = nsb.tile([128, 1], F32, tag="eps", bufs=1)
              nc.vector.memset(eps_t[:], 1e-16)
              for it in range(n_it):
                  pn0 = nps.tile([128, 128], F32, tag="pn0")
                  nc.tensor.transpose(pn0[:], embT[:, 0, it * 128:(it + 1) * 128], ident[:])
                  pn1 = nps.tile([128, 128], F32, tag="pn1")
                  nc.tensor.transpose(pn1[:], embT[:, 1, it * 128:(it + 1) * 128], ident[:])
                  emb_n = nsb.tile([128, H], F32, tag="embn")
                  nc.scalar.activation(emb_n[:, 0:128], pn0[:], AF.Copy)
                  nc.scalar.activation(emb_n[:, 128:256], pn1[:], AF.Copy)
                  sq = nsb.tile([128, H], F32, tag="sq")
                  ss = nsb.tile([128, 1], F32, tag="ss")
                  nc.scalar.activation(sq[:], emb_n[:], AF.Square, accum_out=ss[:])
                  # nrm = sqrt(ss + 1e-16) == max(sqrt(ss), 1e-8) up to rounding
                  nrm = nsb.tile([128, 1], F32, tag="nrm")
                  nc.scalar.activation(nrm[:], ss[:], AF.Sqrt, bias=eps_t[:])
                  inv = nsb.tile([128, 1], F32, tag="inv")
                  nc.vector.reciprocal(inv[:], nrm[:])
                  nc.vector.tensor_scalar(en16_blk[:, it, :], emb_n[:], inv[:], None,
                                          op0=ALU.mult)
                  for dh in range(2):
                      pt = nps.tile([128, 128], F16, tag="pt2")
                      nc.tensor.transpose(
                          pt[:], en16_blk[:, it, dh * 128:(dh + 1) * 128], ident16[:])
                      nc.scalar.activation(
                          enT16_blk[:, dh, it * 128:(it + 1) * 128], pt[:], AF.Copy)
              gin2 = dram.tile([128, 2, blk], F16, tag="eg_in")
              nc.sync.dma_start(gin2[:], enT16_blk[:])
              if fake_collectives:
                  gout2 = dram.tile([N_CORES, 128, 2, blk], F16, tag="eg_out")
                  for cc in range(N_CORES):
                      nc.sync.dma_start(gout2[cc], gin2[:])
              else:
                  gout2 = dram.tile([N_CORES, 128, 2, blk], F16, tag="eg_out",
                                    addr_space="Shared")
                  nc.gpsimd.collective_compute(
                      "AllGather", ALU.bypass, replica_groups=groups,
                      ins=[gin2[:].opt()], outs=[gout2[:].opt()])
              # per-core staging tiles: S1's first chunks only wait for core 0
              enT16_full = []
              for cc in range(N_CORES):
                  en_fc = spool.tile([128, 2, blk], F16, name=f"enf{cc}")
                  nc.sync.dma_start(en_fc[:], gout2[cc])
                  enT16_full.append(en_fc)

          # ---------- S1: per-row top-16 threshold tau ----------
          tau_rep = spool.tile([128, blk], F32)
          with (
              tc.tile_pool(name="sps", bufs=2, space="PSUM") as sps,
              tc.tile_pool(name="t8ps", bufs=2, space="PSUM") as t8ps,
              tc.tile_pool(name="s1sb", bufs=2) as s1sb,
          ):
              tau_col = s1sb.tile([128, n_it], F32, tag="tau_col", bufs=1)
              for it in range(n_it):
                  # top-8 per 1024-chunk (a row whose true top-16 packs >8
                  # into one chunk is ~1e-5 likely; within tolerance), then
                  # exact top-16 of the 128 candidates.
                  cands = s1sb.tile([128, n_chunk * 8], F32, tag="cands")
                  for ch in range(n_chunk):
                      strip = sps.tile([128, chunkw], F32, tag="strip")
                      for st in range(chunkw // subw):
                          j0 = ch * chunkw + st * subw
                          cb, off = divmod(j0, blk)
                          nc.tensor.matmul(
                              strip[:, st * subw:(st + 1) * subw],
                              enT16_blk[:, 0, it * 128:(it + 1) * 128],
                              enT16_full[cb][:, 0, off:off + subw],
                              start=True, stop=False)
                          nc.tensor.matmul(
                              strip[:, st * subw:(st + 1) * subw],
                              enT16_blk[:, 1, it * 128:(it + 1) * 128],
                              enT16_full[cb][:, 1, off:off + subw],
                              start=False, stop=True)
                      nc.vector.max(out=cands[:, ch * 8:ch * 8 + 8], in_=strip[:])
                  m1 = s1sb.tile([128, 8], F32, tag="m1")
                  nc.vector.max(out=m1[:], in_=cands[:])
                  nc.vector.match_replace(out=cands[:], in_to_replace=m1[:],
                                          in_values=cands[:], imm_value=NEG)
                  m2 = s1sb.tile([128, 8], F32, tag="m2")
                  nc.vector.max(out=m2[:], in_=cands[:])
                  nc.vector.tensor_copy(tau_col[:, it:it + 1], m2[:, 7:8])
              # tau_col [128, n_it] -> tauT [n_it, 128] -> row [1, blk] -> tau_rep
              ptt = t8ps.tile([128, 128], F32, tag="ptt")
              nc.tensor.transpose(ptt[:n_it, :], tau_col[:], ident[:])
              tauT = s1sb.tile([n_it, 128], F32, tag="tauT", bufs=1)
              nc.scalar.activation(tauT[:], ptt[:n_it, :], AF.Copy)
              taurow = s1sb.tile([1, blk], F32, tag="taurow", bufs=1)
              nc.sync.dma_start(taurow[:], tauT[:])
              bw = min(512, blk)
              for bb in range(blk // bw):
                  pb = t8ps.tile([128, bw], F32, tag="pb")
                  nc.tensor.matmul(pb[:], ones1[:], taurow[:, bb * bw:(bb + 1) * bw],
                                   start=True, stop=True)
                  nc.scalar.activation(tau_rep[:, bb * bw:(bb + 1) * bw], pb[:], AF.Copy)

          # ---------- S2: fused = (exp(sim) * (sim >= tau)) @ OH; p_sim ----------
          with (
              tc.tile_pool(name="simps", bufs=3, space="PSUM") as simps,
              tc.tile_pool(name="fps", bufs=2, space="PSUM") as fps,
              tc.tile_pool(name="ftps", bufs=2, space="PSUM") as ftps,
              tc.tile_pool(name="s2sb", bufs=3) as s2sb,
          ):
              for ig in range(n_ig):
                  sl = slice(ig * igw, (ig + 1) * igw)
                  pfused = fps.tile([C, igw], F32, tag="pf")
                  for jt in range(n_js):
                      cb, off = divmod(jt * 128, blk)
                      psim = simps.tile([128, igw], F32, tag="psim")
                      nc.tensor.matmul(psim[:], enT16_full[cb][:, 0, off:off + 128],
                                       enT16_blk[:, 0, sl], start=True, stop=False)
                      nc.tensor.matmul(psim[:], enT16_full[cb][:, 1, off:off + 128],
                                       enT16_blk[:, 1, sl], start=False, stop=True)
                      e16 = s2sb.tile([128, igw], F16, tag="e16")
                      nc.scalar.activation(e16[:], psim[:], AF.Exp)
                      mk16 = s2sb.tile([128, igw], F16, tag="mk16")
                      nc.vector.tensor_tensor(mk16[:], psim[:], tau_rep[:, sl],
                                              op=ALU.is_ge)
                      ew16 = s2sb.tile([128, igw], F16, tag="ew16")
                      nc.vector.tensor_mul(ew16[:], e16[:], mk16[:])
                      nc.tensor.matmul(pfused[:], oh_sb[:, jt, :], ew16[:],
                                       start=(jt == 0), stop=(jt == n_js - 1))
                  fsb = s2sb.tile([C, igw], F32, tag="fsb")
                  nc.scalar.activation(fsb[:], pfused[:], AF.Copy)
                  for t in range(igw // 128):
                      it = ig * (igw // 128) + t
                      pft = ftps.tile([128, C], F32, tag="pft")
                      nc.tensor.transpose(pft[:, :C], fsb[:, t * 128:(t + 1) * 128],
                                          ident[:C, :C])
                      logsoftmax_from_psum(out_acc[:, it, :], pft[:, :C], s2sb,
                                           add_into=out_acc[:, it, :])
        nc.sync.dma_start(out_d.ap().rearrange("(it p) c -> p it c", p=128),
                          out_acc[:])

    nc.compile()
    return nc


def prep_inputs(inputs, n=N, n_layers=N_LAYERS):
    """Host-side sharding/layout prep. Returns in_maps (one dict per core)."""
    blk = n // N_CORES
    igw = min(512, blk)
    n_ig = blk // igw
    n_js = n // 128
    n_bkt = n_js * n_ig
    x = np.asarray(inputs["x"], np.float32)
    y = np.asarray(inputs["y"]).astype(np.int64)
    ei = np.asarray(inputs["edge_index"]).astype(np.int64)
    ew = np.asarray(inputs["edge_weight"], np.float32)
    src, dst = ei[0], ei[1]

    y32 = np.ascontiguousarray(
        y.reshape(n_js, 128).T.astype(np.float16))  # [128, n_js]
    w_in16 = np.ascontiguousarray(
        np.asarray(inputs["W_in"], np.float32)
        .reshape(D_IN // 128, 128, H).transpose(1, 0, 2)).astype(np.float16)
    b_in16 = np.asarray(inputs["b_in"], np.float32).reshape(1, H).astype(np.float16)
    w_out = np.ascontiguousarray(
        np.asarray(inputs["W_out"], np.float32)
        .reshape(2, 128, C).transpose(1, 0, 2))
    b_out = np.asarray(inputs["b_out"], np.float32).reshape(1, C)
    cw116 = np.ascontiguousarray(
        np.asarray(inputs["conv_w1"], np.float32)
        .reshape(n_layers, 2, 128, H).transpose(0, 2, 1, 3)).astype(np.float16)
    cw216 = np.ascontiguousarray(
        np.asarray(inputs["conv_w2"], np.float32)
        .reshape(n_layers, 2, 128, H).transpose(0, 2, 1, 3)).astype(np.float16)

    n_aw = blk // AW
    n_bkt = n_js * n_aw
    core_of = dst // blk
    in_maps = []
    for c in range(N_CORES):
        sel = core_of == c
        s, d, w = src[sel], dst[sel] - c * blk, ew[sel]
        js, sl_ = s >> 7, s & 127
        aw_, dw = d // AW, d % AW
        bkt = js * n_aw + aw_
        order = np.argsort(bkt, kind="stable")
        bs = bkt[order]
        counts = np.bincount(bs, minlength=n_bkt)
        if counts.max() > CAP:
            raise RuntimeError(f"edge bucket overflow: {counts.max()} > {CAP}")
        starts = np.zeros(n_bkt + 1, np.int64)
        np.cumsum(counts, out=starts[1:])
        pos = np.arange(len(bs)) - starts[bs]
        esrc_a = np.full((n_bkt, CAP), -1.0, np.float16)
        edst_a = np.full((n_bkt, CAP), -1.0, np.float16)
        ew_a = np.zeros((n_bkt, CAP), np.float16)
        esrc_a[bs, pos] = sl_[order]
        edst_a[bs, pos] = dw[order]
        ew_a[bs, pos] = w[order]
        # [n_js, n_aw, CAP=128] -> [128, n_js, n_aw]
        def to_dev(a):
            return np.ascontiguousarray(
                a.reshape(n_js, n_aw, 128).transpose(2, 0, 1))
        lo, hi = c * blk, (c + 1) * blk
        xT16 = np.ascontiguousarray(
            x[lo:hi].T.reshape(D_IN // 128, 128, blk)
            .transpose(1, 0, 2)).astype(np.float16)
        in_maps.append({
            "xT16": xT16, "esrc": to_dev(esrc_a), "edst": to_dev(edst_a),
            "ew": to_dev(ew_a), "y_r": y32, "w_in16": w_in16, "b_in16": b_in16,
            "cw116": cw116, "cw216": cw216, "w_out_r": w_out, "b_out_r": b_out,
        })
    return in_maps


def make_device_runner(nc, n_cores=N_CORES, chain=1):
    """Persistent PJRT executor mirroring bass2jax.run_bass_via_pjrt.

    Returns (upload, put_zeros, run, out_names): upload() transfers fresh
    host inputs to the 8 cores, put_zeros() stages the donated output
    buffers, run() executes the NEFF and blocks until complete.

    chain > 1 executes the NEFF `chain` times inside one dispatch, feeding
    each run's outputs in as the next run's output operands (bass_exec's
    ordered effect + the data dependency force strict sequencing) — used to
    amortize the host<->device dispatch RTT out of timing measurements.
    """
    import jax
    from jax.experimental.shard_map import shard_map
    from jax.sharding import Mesh, NamedSharding, PartitionSpec
    from concourse.bass2jax import (_bass_exec_p, install_neuronx_cc_hook,
                                    partition_id_tensor)

    install_neuronx_cc_hook()
    partition_name = nc.partition_id_tensor.name if nc.partition_id_tensor else None

    in_names, out_names, out_avals, zero_outs = [], [], [], []
    for alloc in nc.m.functions[0].allocations:
        if not isinstance(alloc, mybir.MemoryLocationSet):
            continue
        name = alloc.memorylocations[0].name
        if alloc.kind == "ExternalInput":
            if name != partition_name:
                in_names.append(name)
        elif alloc.kind == "ExternalOutput":
            shape = tuple(alloc.tensor_shape)
            dtype = mybir.dt.np(alloc.dtype)
            out_names.append(name)
            out_avals.append(jax.core.ShapedArray(shape, dtype))
            zero_outs.append(np.zeros(shape, dtype))
    n_params = len(in_names)
    n_outs = len(out_avals)
    all_in_names = list(in_names) + out_names
    if partition_name is not None:
        all_in_names.append(partition_name)
    donate = tuple(range(n_params, n_params + n_outs))

    def _body(*args):
        ins = list(args[:n_params])
        zs = list(args[n_params:])
        for _ in range(chain):
            operands = ins + zs
            if partition_name is not None:
                operands.append(partition_id_tensor())
            zs = list(_bass_exec_p.bind(
                *operands,
                out_avals=tuple(out_avals),
                in_names=tuple(all_in_names),
                out_names=tuple(out_names),
                lowering_input_output_aliases=(),
                sim_require_finite=True,
                sim_require_nnan=True,
                nc=nc,
            ))
        return tuple(zs)

    devices = jax.devices()[:n_cores]
    mesh = Mesh(np.asarray(devices), ("core",))
    in_specs = (PartitionSpec("core"),) * (n_params + n_outs)
    out_specs = (PartitionSpec("core"),) * n_outs
    sharded = jax.jit(
        shard_map(_body, mesh=mesh, in_specs=in_specs, out_specs=out_specs,
                  check_rep=False),
        donate_argnums=donate, keep_unused=True)
    sh = NamedSharding(mesh, PartitionSpec("core"))
    zero_shapes = [(n_cores * z.shape[0], *z.shape[1:]) for z in zero_outs]
    zero_dtypes = [z.dtype for z in zero_outs]

    def upload(in_maps):
        arrs = [
            jax.device_put(
                np.concatenate(
                    [np.asarray(in_maps[c][name]) for c in range(n_cores)],
                    axis=0), sh)
            for name in in_names
        ]
        jax.block_until_ready(arrs)
        return arrs

    def put_zeros():
        zs = [jax.device_put(np.zeros(s, d), sh)
              for s, d in zip(zero_shapes, zero_dtypes)]
        jax.block_until_ready(zs)
        return zs

    def run(dev_in, zs, block=True):
        outs = sharded(*dev_in, *zs)
        if block:
            jax.block_until_ready(outs)
        return outs

    return upload, put_zeros, run, out_names


_CACHED_NC = None
_CACHED_RUNNER = None


def kernel(**inputs):
    global _CACHED_NC, _CACHED_RUNNER
    if _CACHED_NC is None:
        _CACHED_NC = build_program()
        _CACHED_RUNNER = make_device_runner(_CACHED_NC)
    upload, put_zeros, run, out_names = _CACHED_RUNNER
    in_maps = prep_inputs(inputs)
    dev_in = upload(in_maps)
    zs = put_zeros()
    outs = run(dev_in, zs)
    oi = out_names.index("out")
    blk = N // N_CORES
    out = np.asarray(outs[oi]).reshape(N, C)
    return out.astype(np.float32)


if __name__ == "__main__":
    nc = build_program()
    print("built + compiled OK")



# revision 35
# speedup vs baseline: 1.0642x; 1.0642x over previous
"""GCNII encoder + KNN label-fusion subgraph on 8 Trainium2 NeuronCores.

Sharding: nodes (rows) split into 8 blocks of N/8. Each core:
  - builds its dense fp16 adjacency block A^T[:, blk] ON DEVICE from a
    compact padded COO edge list (iota-compare one-hots + PE matmuls),
    so only ~6 MiB/core of inputs cross the host link instead of 512 MiB
  - computes h = relu(x_blk @ W_in + b_in)
  - 9 GCNII layers: agg_blk = A[blk, :] @ h_full (dense fp16 adjacency
    streamed from device HBM as PE matmuls), h_full re-AllGathered (fp16)
  - p_lc = log_softmax(emb @ W_out + b_out) on its rows
  - cosine-sim branch: en = emb/||emb||; per-row exact top-16 threshold tau
    via max8/match_replace8 over PSUM sim strips; fused = (exp(sim) *
    (sim >= tau)) @ one_hot(y) as PE matmuls; p_sim = log_softmax(fused)
  - out = 0.5*p_lc + 0.5*p_sim
Host only preps layouts: bucketed edge lists, transposed x, weights.
"""
import math
from contextlib import ExitStack

import numpy as np

import concourse.bass as bass
import concourse.tile as tile
from concourse import bacc, mybir
from concourse.bass_utils import run_bass_kernel_spmd
from concourse.masks import make_identity

F32 = mybir.dt.float32
F16 = mybir.dt.float16
I32 = mybir.dt.int32
AF = mybir.ActivationFunctionType
ALU = mybir.AluOpType

N_CORES = 8
N = 16384
D_IN = 512
H = 256
C = 64
K_TOP = 16
N_LAYERS = 9
ALPHA = 0.5
THETA = 1.0
NEG = -1e30

# edge bucketing: bucket = (src slab js, 256-wide dst window a), one 128-edge
# chunk per bucket. Mean fill 64, CAP 128 = +8 sigma; overflow ~impossible.
AW = 256             # dst window width per bucket
CAP = 128            # bucket capacity (one matmul chunk)
GW = 16              # A-build: js slabs batched per at_dram write DMA
GR = 8               # agg: js slabs batched per at_dram read DMA


def _betas():
    return [float(np.log(THETA / (l + 1) + 1.0)) for l in range(N_LAYERS)]


def build_program(n=N, n_layers=N_LAYERS, skip_abuild=False, skip_sim=False,
                  fake_collectives=False, abuild_mode='vec'):
    blk = n // N_CORES          # rows per core
    n_it = blk // 128           # 128-row tiles per block
    igw = min(512, blk)         # i-group width (dst cols per psum tile)
    n_ig = blk // igw
    n_js = n // 128             # src slabs
    chunkw = min(1024, n)       # S1 scan chunk width
    n_chunk = n // chunkw
    subw = min(512, blk)        # sim rhs tile width (<= c-block, <= 512)
    betas = _betas()

    nc = bacc.Bacc("TRN2", target_bir_lowering=False, debug=False,
                   num_devices=N_CORES)

    n_aw = blk // AW            # dst windows per core
    xT_d = nc.dram_tensor("xT16", [128, D_IN // 128, blk], F16,
                          kind="ExternalInput")
    esrc_d = nc.dram_tensor("esrc", [128, n_js, n_aw], F16,
                            kind="ExternalInput")
    edst_d = nc.dram_tensor("edst", [128, n_js, n_aw], F16,
                            kind="ExternalInput")
    ew_d = nc.dram_tensor("ew", [128, n_js, n_aw], F16,
                          kind="ExternalInput")
    y_d = nc.dram_tensor("y_r", [128, n_js], F16, kind="ExternalInput")
    w_in_d = nc.dram_tensor("w_in16", [128, D_IN // 128, H], F16,
                            kind="ExternalInput")
    b_in_d = nc.dram_tensor("b_in16", [1, H], F16, kind="ExternalInput")
    cw1_d = nc.dram_tensor("cw116", [N_LAYERS, 128, 2, H], F16,
                           kind="ExternalInput")
    cw2_d = nc.dram_tensor("cw216", [N_LAYERS, 128, 2, H], F16,
                           kind="ExternalInput")
    w_out_d = nc.dram_tensor("w_out_r", [128, 2, C], F32, kind="ExternalInput")
    b_out_d = nc.dram_tensor("b_out_r", [1, C], F32, kind="ExternalInput")
    out_d = nc.dram_tensor("out", [blk, C], F32, kind="ExternalOutput")

    groups = [list(range(N_CORES))]

    with tile.TileContext(nc) as tc, ExitStack() as S:
        const = S.enter_context(tc.tile_pool(name="const", bufs=1))
        dram = S.enter_context(tc.tile_pool(name="dram", bufs=1, space="DRAM"))
        hT_pool = S.enter_context(tc.tile_pool(name="hTp", bufs=2))
        # GCN-phase pools, released before the similarity phase
        G = ExitStack()
        x0pool = G.enter_context(tc.tile_pool(name="x0p", bufs=1))
        hfull_pool = G.enter_context(tc.tile_pool(name="hfp", bufs=1))
        h16b_pool = G.enter_context(tc.tile_pool(name="h16bp", bufs=2))

        ident = const.tile([128, 128], F32)
        make_identity(nc, ident[:])
        ident16 = const.tile([128, 128], F16)
        nc.vector.tensor_copy(ident16[:], ident[:])
        ones1 = const.tile([1, 128], F32)
        nc.vector.memset(ones1[:], 1.0)
        ones1_16 = const.tile([1, 128], F16)
        nc.vector.memset(ones1_16[:], 1.0)
        # iota tiles for one-hot construction (values exact in f16)
        iota_w_i = const.tile([128, igw], I32)
        nc.gpsimd.iota(iota_w_i[:], pattern=[[1, igw]], channel_multiplier=0)
        iota_w16 = const.tile([128, igw], F16)
        nc.vector.tensor_copy(iota_w16[:], iota_w_i[:])
        iota_p_i = const.tile([128, 128], I32)
        nc.gpsimd.iota(iota_p_i[:], pattern=[[1, 128]], channel_multiplier=0)
        iota_p16 = const.tile([128, 128], F16)
        nc.vector.tensor_copy(iota_p16[:], iota_p_i[:])
        iota_c = const.tile([128, C], F32)
        nc.vector.tensor_copy(iota_c[:], iota_p_i[:, :C])

        w_in_sb = const.tile([128, D_IN // 128, H], F16)
        nc.sync.dma_start(w_in_sb[:], w_in_d.ap())
        b_in_sb = const.tile([1, H], F16)
        nc.sync.dma_start(b_in_sb[:], b_in_d.ap())
        w_out_sb = const.tile([128, 2, C], F32)
        nc.sync.dma_start(w_out_sb[:], w_out_d.ap())
        b_out_sb = const.tile([1, C], F32)
        nc.sync.dma_start(b_out_sb[:], b_out_d.ap())

        # ablation-only: keep every NEFF input bound when a phase is skipped
        if skip_abuild:
            dmy = const.tile([128, n_js, n_aw], F16)
            nc.sync.dma_start(dmy[:], esrc_d.ap())
            dmy2 = const.tile([128, n_js, n_aw], F16)
            nc.sync.dma_start(dmy2[:], edst_d.ap())
            dmy3 = const.tile([128, n_js, n_aw], F16)
            nc.sync.dma_start(dmy3[:], ew_d.ap())
        if n_layers == 0:
            dmy4 = const.tile([128, 2, H], F16)
            nc.sync.dma_start(dmy4[:], cw1_d.ap()[0])
            dmy5 = const.tile([128, 2, H], F16)
            nc.sync.dma_start(dmy5[:], cw2_d.ap()[0])

        # ---------- y labels (one-hot built later, in the sim phase) ----------
        y16_sb = const.tile([128, n_js], F16)
        nc.sync.dma_start(y16_sb[:], y_d.ap())
        y_sb = const.tile([128, n_js], F32)
        nc.vector.tensor_copy(y_sb[:], y16_sb[:])

        x0sT = x0pool.tile([128, 2, blk], F32)
        x0sT16 = x0pool.tile([128, 2, blk], F16)
        out_acc = const.tile([128, n_it, C], F32)

        def logsoftmax_from_psum(dst_ap, psrc, sp, add_into=None):
            """dst = 0.5 * log_softmax(psrc rows); psrc is [128, C] psum."""
            m8 = sp.tile([128, 8], F32, tag="ls_m")
            nc.vector.max(out=m8[:], in_=psrc[:])
            m = m8[:, 0:1]
            mneg = sp.tile([128, 1], F32, tag="ls_mn")
            nc.vector.tensor_scalar_mul(mneg[:], m, -1.0)
            e = sp.tile([128, C], F32, tag="ls_e")
            ssum = sp.tile([128, 1], F32, tag="ls_s")
            nc.scalar.activation(e[:], psrc[:], AF.Exp, bias=mneg[:], scale=1.0,
                                 accum_out=ssum[:])
            ls = sp.tile([128, 1], F32, tag="ls_l")
            nc.scalar.activation(ls[:], ssum[:], AF.Ln)
            m2 = sp.tile([128, 1], F32, tag="ls_m2")
            nc.vector.tensor_add(m2[:], m, ls[:])
            if add_into is None:
                nc.vector.tensor_scalar(dst_ap, psrc[:], m2[:], 0.5,
                                        op0=ALU.subtract, op1=ALU.mult)
            else:
                t = sp.tile([128, C], F32, tag="ls_t")
                nc.vector.tensor_scalar(t[:], psrc[:], m2[:], 0.5,
                                        op0=ALU.subtract, op1=ALU.mult)
                nc.vector.tensor_add(dst_ap, add_into, t[:])

        def allgather_h16(h16_blk_t, tag):
            gin = dram.tile([128, n_it, H], F16, tag=f"{tag}_in")
            nc.sync.dma_start(gin[:], h16_blk_t[:])
            if fake_collectives:
                gout = dram.tile([N_CORES, 128, n_it, H], F16,
                                 tag=f"{tag}_out")
                for cc in range(N_CORES):
                    nc.sync.dma_start(gout[cc], gin[:])
            else:
                gout = dram.tile([N_CORES, 128, n_it, H], F16,
                                 tag=f"{tag}_out", addr_space="Shared")
                nc.gpsimd.collective_compute(
                    "AllGather", ALU.bypass, replica_groups=groups,
                    ins=[gin[:].opt()], outs=[gout[:].opt()])
            # one SBUF tile per core so agg matmuls on core jc's slabs only
            # wait for that core's copy, not the whole 8 MiB staging
            hf = []
            for cc in range(N_CORES):
                hf_c = hfull_pool.tile([128, n_it, H], F16, tag=f"hfull{cc}",
                                       name=f"hfull{cc}")
                nc.sync.dma_start(hf_c[:], gout[cc])
                hf.append(hf_c)
            return hf

        # ---------- phase 0: h0 = relu(x @ W_in + b_in) ----------
        with (
            tc.tile_pool(name="p0ps", bufs=2, space="PSUM") as p0ps,
            tc.tile_pool(name="p0sb", bufs=3) as p0sb,
            tc.tile_pool(name="p0x", bufs=1) as p0x,
        ):
            xT_sb = p0x.tile([128, D_IN // 128, blk], F16)
            nc.sync.dma_start(xT_sb[:], xT_d.ap())
            hT = hT_pool.tile([128, 2, blk], F32, tag="hT")
            h16_blk = h16b_pool.tile([128, n_it, H], F16, tag="h16b")
            for it in range(n_it):
                ph = p0ps.tile([128, H], F32, tag="ph")
                for k in range(D_IN // 128):
                    nc.tensor.matmul(ph[:], xT_sb[:, k, it * 128:(it + 1) * 128],
                                     w_in_sb[:, k, :], start=(k == 0), stop=False)
                nc.tensor.matmul(ph[:], ones1_16[:], b_in_sb[:], start=False,
                                 stop=True)
                hm = p0sb.tile([128, H], F32, tag="hm")
                nc.scalar.activation(hm[:], ph[:], AF.Relu)
                nc.vector.tensor_copy(h16_blk[:, it, :], hm[:])
                for dh in range(2):
                    pt = p0ps.tile([128, 128], F32, tag="pt")
                    nc.tensor.transpose(pt[:], hm[:, dh * 128:(dh + 1) * 128], ident[:])
                    nc.scalar.activation(hT[:, dh, it * 128:(it + 1) * 128], pt[:], AF.Copy)
            nc.vector.tensor_scalar_mul(x0sT[:], hT[:], 0.5)
            nc.vector.tensor_copy(x0sT16[:], x0sT[:])
        h16_full = allgather_h16(h16_blk, "ag")

        # ---------- build dense A^T on device from bucketed COO ----------
        # at_dram[ig][p, js, j] = sum over edges (src=js*128+p -> dst ig*igw+j)
        # partition-major so reads/writes batch GW/GR slabs per DMA with one
        # contiguous descriptor per partition; split per ig so layer-0 agg on
        # stripe ig can start as soon as that stripe's build completes.
        at_dram = []
        for ig in range(n_ig):
            at_ig = dram.tile([128, n_js, igw], F16, name=f"at_ig{ig}")
            at_dram.append(at_ig)
        if not skip_abuild:
          with (
            tc.tile_pool(name="abps", bufs=4, space="PSUM") as abps,
            tc.tile_pool(name="absb", bufs=3) as absb,
            tc.tile_pool(name="abwr", bufs=2) as abwr,
            tc.tile_pool(name="aesb", bufs=1) as aesb,
          ):
            esrc16 = aesb.tile([128, n_js, n_aw], F16)
            nc.sync.dma_start(esrc16[:], esrc_d.ap())
            edst16 = aesb.tile([128, n_js, n_aw], F16)
            nc.sync.dma_start(edst16[:], edst_d.ap())
            ew16_t = aesb.tile([128, n_js, n_aw], F16)
            nc.sync.dma_start(ew16_t[:], ew_d.ap())
            esrc_sb = aesb.tile([128, n_js, n_aw], F32)
            nc.vector.tensor_copy(esrc_sb[:], esrc16[:])
            edst_sb = aesb.tile([128, n_js, n_aw], F32)
            nc.vector.tensor_copy(edst_sb[:], edst16[:])
            ew_sb = aesb.tile([128, n_js, n_aw], F32)
            nc.vector.tensor_copy(ew_sb[:], ew16_t[:])
            npw = igw // AW  # buckets per igw stripe
            for ig in range(n_ig):
                for jg in range(n_js // GW):
                    wr = abwr.tile([128, GW, igw], F16, tag="wr")
                    for jl in range(GW):
                        js = jg * GW + jl
                        for a2 in range(npw):
                            a = ig * npw + a2
                            pa = abps.tile([128, AW], F32, tag="pa")
                            ohs = absb.tile([128, 128], F16, tag="ohs")
                            nc.vector.tensor_scalar(
                                ohs[:], iota_p16[:], esrc_sb[:, js, a:a + 1],
                                None, op0=ALU.is_equal)
                            ohd = absb.tile([128, AW], F16, tag="ohd")
                            nc.vector.tensor_scalar(
                                ohd[:], iota_w16[:, :AW], edst_sb[:, js, a:a + 1],
                                ew_sb[:, js, a:a + 1],
                                op0=ALU.is_equal, op1=ALU.mult)
                            nc.tensor.matmul(pa[:], ohs[:], ohd[:],
                                             start=True, stop=True)
                            # alternate copy engine: ACT and DVE share the load
                            if (jl + a2) % 2 == 0:
                                nc.scalar.activation(
                                    wr[:, jl, a2 * AW:(a2 + 1) * AW], pa[:], AF.Copy)
                            else:
                                nc.vector.tensor_copy(
                                    wr[:, jl, a2 * AW:(a2 + 1) * AW], pa[:])
                    nc.sync.dma_start(at_dram[ig][:, jg * GW:(jg + 1) * GW, :],
                                      wr[:])

        # ---------- GCN layers ----------
        with (
            tc.tile_pool(name="aggps", bufs=2, space="PSUM") as aggps,
            tc.tile_pool(name="mmps", bufs=2, space="PSUM") as mmps,
            tc.tile_pool(name="tps", bufs=2, space="PSUM") as tps,
            tc.tile_pool(name="apool", bufs=3) as apool,
            tc.tile_pool(name="wpool", bufs=2) as wpool,
            tc.tile_pool(name="xpool", bufs=2) as xpool,
            tc.tile_pool(name="tpool", bufs=3) as tpool,
        ):
            for l in range(n_layers):
                beta = betas[l]
                cw1_sb = wpool.tile([128, 2, H], F16, tag="cw1")
                nc.sync.dma_start(cw1_sb[:], cw1_d.ap()[l])
                cw2_sb = wpool.tile([128, 2, H], F16, tag="cw2")
                nc.sync.dma_start(cw2_sb[:], cw2_d.ap()[l])
                hT_new = hT_pool.tile([128, 2, blk], F32, tag="hT")
                for ig in range(n_ig):
                    pa0 = aggps.tile([128, igw], F32, tag="agg0")
                    pa1 = aggps.tile([128, igw], F32, tag="agg1")
                    for g in range(n_js // GR):
                        a_t = apool.tile([128, GR, igw], F16, tag="a")
                        nc.sync.dma_start(a_t[:],
                                          at_dram[ig][:, g * GR:(g + 1) * GR, :])
                        for r in range(GR):
                            js = g * GR + r
                            jc, jb = divmod(js, n_it)
                            nc.tensor.matmul(pa0[:], h16_full[jc][:, jb, 0:128],
                                             a_t[:, r, :], start=(js == 0),
                                             stop=(js == n_js - 1))
                            nc.tensor.matmul(pa1[:], h16_full[jc][:, jb, 128:256],
                                             a_t[:, r, :], start=(js == 0),
                                             stop=(js == n_js - 1))
                    xsT = xpool.tile([128, 2, igw], F32, tag="xsT")
                    nc.scalar.activation(xsT[:, 0, :], pa0[:], AF.Copy, scale=0.5)
                    nc.scalar.activation(xsT[:, 1, :], pa1[:], AF.Copy, scale=0.5)
                    xsT16 = xpool.tile([128, 2, igw], F16, tag="xsT16")
                    nc.vector.tensor_copy(xsT16[:], xsT[:])
                    sl = slice(ig * igw, (ig + 1) * igw)
                    for dh in range(2):
                        # cw2 terms first: they only need x0sT16, so the PE can
                        # start them while the AllGather/agg is still in flight
                        pmm = mmps.tile([128, igw], F32, tag="pmm")
                        nc.tensor.matmul(pmm[:], cw2_sb[:, 0, dh * 128:(dh + 1) * 128],
                                         x0sT16[:, 0, sl], start=True, stop=False)
                        nc.tensor.matmul(pmm[:], cw2_sb[:, 1, dh * 128:(dh + 1) * 128],
                                         x0sT16[:, 1, sl], start=False, stop=False)
                        nc.tensor.matmul(pmm[:], cw1_sb[:, 0, dh * 128:(dh + 1) * 128],
                                         xsT16[:, 0, :], start=False, stop=False)
                        nc.tensor.matmul(pmm[:], cw1_sb[:, 1, dh * 128:(dh + 1) * 128],
                                         xsT16[:, 1, :], start=False, stop=True)
                        t1 = tpool.tile([128, igw], F32, tag="t1")
                        nc.vector.tensor_add(t1[:], xsT[:, dh, :], x0sT[:, dh, sl])
                        t2 = tpool.tile([128, igw], F32, tag="t2")
                        nc.scalar.activation(t2[:], pmm[:], AF.Copy, scale=beta)
                        nc.vector.tensor_scalar_mul(t1[:], t1[:], 1.0 - beta)
                        nc.vector.tensor_add(t1[:], t1[:], t2[:])
                        nc.vector.tensor_add(t1[:], t1[:], hT[:, dh, sl])
                        nc.scalar.activation(hT_new[:, dh, sl], t1[:], AF.Relu)
                hT = hT_new
                if l < n_layers - 1:
                    h16_new = h16b_pool.tile([128, n_it, H], F16, tag="h16b")
                    for it in range(n_it):
                        for dh in range(2):
                            pt = tps.tile([128, 128], F32, tag="pt")
                            nc.tensor.transpose(
                                pt[:], hT[:, dh, it * 128:(it + 1) * 128], ident[:])
                            nc.scalar.activation(
                                h16_new[:, it, dh * 128:(dh + 1) * 128], pt[:], AF.Copy)
                    h16_full = allgather_h16(h16_new, "ag")
        embT = hT  # [128, 2, blk] f32
        G.close()  # release GCN-phase SBUF (h16_full, x0sT, h16_blk)
        spool = S.enter_context(tc.tile_pool(name="spool", bufs=1))

        # ---------- p_lc ----------
        with (
            tc.tile_pool(name="lcps", bufs=2, space="PSUM") as lcps,
            tc.tile_pool(name="lcsb", bufs=2) as lcsb,
        ):
            for it in range(n_it):
                plc = lcps.tile([128, C], F32, tag="plc")
                nc.tensor.matmul(plc[:], embT[:, 0, it * 128:(it + 1) * 128],
                                 w_out_sb[:, 0, :], start=True, stop=False)
                nc.tensor.matmul(plc[:], embT[:, 1, it * 128:(it + 1) * 128],
                                 w_out_sb[:, 1, :], start=False, stop=False)
                nc.tensor.matmul(plc[:], ones1[:], b_out_sb[:], start=False, stop=True)
                logsoftmax_from_psum(out_acc[:, it, :], plc, lcsb)

        if not skip_sim:
          # ---------- one-hot(y) (deferred from init to keep SBUF free) ----------
          oh_sb = spool.tile([128, n_js, C], F16)
          for js in range(n_js):
              nc.vector.tensor_scalar(oh_sb[:, js, :], iota_c[:],
                                      y_sb[:, js:js + 1], None, op0=ALU.is_equal)
          # ---------- normalize ----------
          enT16_blk = spool.tile([128, 2, blk], F16)
          with (
              tc.tile_pool(name="nps", bufs=2, space="PSUM") as nps,
              tc.tile_pool(name="nsb", bufs=3) as nsb,
          ):
              en16_blk = nsb.tile([128, n_it, H], F16, tag="en16b", bufs=1)
              eps_t = nsb.tile([128, 1], F32, tag="eps", bufs=1)
              nc.vector.memset(eps_t[:], 1e-16)
              for it in range(n_it):
                  pn0 = nps.tile([128, 128], F32, tag="pn0")
                  nc.tensor.transpose(pn0[:], embT[:, 0, it * 128:(it + 1) * 128], ident[:])
                  pn1 = nps.tile([128, 128], F32, tag="pn1")
                  nc.tensor.transpose(pn1[:], embT[:, 1, it * 128:(it + 1) * 128], ident[:])
                  emb_n = nsb.tile([128, H], F32, tag="embn")
                  nc.scalar.activation(emb_n[:, 0:128], pn0[:], AF.Copy)
                  nc.scalar.activation(emb_n[:, 128:256], pn1[:], AF.Copy)
                  sq = nsb.tile([128, H], F32, tag="sq")
                  ss = nsb.tile([128, 1], F32, tag="ss")
                  nc.scalar.activation(sq[:], emb_n[:], AF.Square, accum_out=ss[:])
                  # nrm = sqrt(ss + 1e-16) == max(sqrt(ss), 1e-8) up to rounding
                  nrm = nsb.tile([128, 1], F32, tag="nrm")
                  nc.scalar.activation(nrm[:], ss[:], AF.Sqrt, bias=eps_t[:])
                  inv = nsb.tile([128, 1], F32, tag="inv")
                  nc.vector.reciprocal(inv[:], nrm[:])
                  nc.vector.tensor_scalar(en16_blk[:, it, :], emb_n[:], inv[:], None,
                                          op0=ALU.mult)
                  for dh in range(2):
                      pt = nps.tile([128, 128], F16, tag="pt2")
                      nc.tensor.transpose(
                          pt[:], en16_blk[:, it, dh * 128:(dh + 1) * 128], ident16[:])
                      nc.scalar.activation(
                          enT16_blk[:, dh, it * 128:(it + 1) * 128], pt[:], AF.Copy)
              gin2 = dram.tile([128, 2, blk], F16, tag="eg_in")
              nc.sync.dma_start(gin2[:], enT16_blk[:])
              if fake_collectives:
                  gout2 = dram.tile([N_CORES, 128, 2, blk], F16, tag="eg_out")
                  for cc in range(N_CORES):
                      nc.sync.dma_start(gout2[cc], gin2[:])
              else:
                  gout2 = dram.tile([N_CORES, 128, 2, blk], F16, tag="eg_out",
                                    addr_space="Shared")
                  nc.gpsimd.collective_compute(
                      "AllGather", ALU.bypass, replica_groups=groups,
                      ins=[gin2[:].opt()], outs=[gout2[:].opt()])
              # per-core staging tiles: S1's first chunks only wait for core 0
              enT16_full = []
              for cc in range(N_CORES):
                  en_fc = spool.tile([128, 2, blk], F16, name=f"enf{cc}")
                  nc.sync.dma_start(en_fc[:], gout2[cc])
                  enT16_full.append(en_fc)

          # ---------- S1: per-row top-16 threshold tau ----------
          tau_rep = spool.tile([128, blk], F32)
          with (
              tc.tile_pool(name="sps", bufs=2, space="PSUM") as sps,
              tc.tile_pool(name="t8ps", bufs=2, space="PSUM") as t8ps,
              tc.tile_pool(name="s1sb", bufs=2) as s1sb,
          ):
              tau_col = s1sb.tile([128, n_it], F32, tag="tau_col", bufs=1)
              for it in range(n_it):
                  # top-8 per 1024-chunk (a row whose true top-16 packs >8
                  # into one chunk is ~1e-5 likely; within tolerance), then
                  # exact top-16 of the 128 candidates.
                  cands = s1sb.tile([128, n_chunk * 8], F32, tag="cands")
                  for ch in range(n_chunk):
                      strip = sps.tile([128, chunkw], F32, tag="strip")
                      for st in range(chunkw // subw):
                          j0 = ch * chunkw + st * subw
                          cb, off = divmod(j0, blk)
                          nc.tensor.matmul(
                              strip[:, st * subw:(st + 1) * subw],
                              enT16_blk[:, 0, it * 128:(it + 1) * 128],
                              enT16_full[cb][:, 0, off:off + subw],
                              start=True, stop=False)
                          nc.tensor.matmul(
                              strip[:, st * subw:(st + 1) * subw],
                              enT16_blk[:, 1, it * 128:(it + 1) * 128],
                              enT16_full[cb][:, 1, off:off + subw],
                              start=False, stop=True)
                      nc.vector.max(out=cands[:, ch * 8:ch * 8 + 8], in_=strip[:])
                  m1 = s1sb.tile([128, 8], F32, tag="m1")
                  nc.vector.max(out=m1[:], in_=cands[:])
                  nc.vector.match_replace(out=cands[:], in_to_replace=m1[:],
                                          in_values=cands[:], imm_value=NEG)
                  m2 = s1sb.tile([128, 8], F32, tag="m2")
                  nc.vector.max(out=m2[:], in_=cands[:])
                  nc.vector.tensor_copy(tau_col[:, it:it + 1], m2[:, 7:8])
              # tau_col [128, n_it] -> tauT [n_it, 128] -> row [1, blk] -> tau_rep
              ptt = t8ps.tile([128, 128], F32, tag="ptt")
              nc.tensor.transpose(ptt[:n_it, :], tau_col[:], ident[:])
              tauT = s1sb.tile([n_it, 128], F32, tag="tauT", bufs=1)
              nc.scalar.activation(tauT[:], ptt[:n_it, :], AF.Copy)
              taurow = s1sb.tile([1, blk], F32, tag="taurow", bufs=1)
              nc.sync.dma_start(taurow[:], tauT[:])
              bw = min(512, blk)
              for bb in range(blk // bw):
                  pb = t8ps.tile([128, bw], F32, tag="pb")
                  nc.tensor.matmul(pb[:], ones1[:], taurow[:, bb * bw:(bb + 1) * bw],
                                   start=True, stop=True)
                  nc.scalar.activation(tau_rep[:, bb * bw:(bb + 1) * bw], pb[:], AF.Copy)

          # ---------- S2: fused = (exp(sim) * (sim >= tau)) @ OH; p_sim ----------
          with (
              tc.tile_pool(name="simps", bufs=3, space="PSUM") as simps,
              tc.tile_pool(name="fps", bufs=2, space="PSUM") as fps,
              tc.tile_pool(name="ftps", bufs=2, space="PSUM") as ftps,
              tc.tile_pool(name="s2sb", bufs=3) as s2sb,
          ):
              for ig in range(n_ig):
                  sl = slice(ig * igw, (ig + 1) * igw)
                  pfused = fps.tile([C, igw], F32, tag="pf")
                  for jt in range(n_js):
                      cb, off = divmod(jt * 128, blk)
                      psim = simps.tile([128, igw], F32, tag="psim")
                      nc.tensor.matmul(psim[:], enT16_full[cb][:, 0, off:off + 128],
                                       enT16_blk[:, 0, sl], start=True, stop=False)
                      nc.tensor.matmul(psim[:], enT16_full[cb][:, 1, off:off + 128],
                                       enT16_blk[:, 1, sl], start=False, stop=True)
                      e16 = s2sb.tile([128, igw], F16, tag="e16")
                      nc.scalar.activation(e16[:], psim[:], AF.Exp)
                      mk16 = s2sb.tile([128, igw], F16, tag="mk16")
                      nc.vector.tensor_tensor(mk16[:], psim[:], tau_rep[:, sl],
                                              op=ALU.is_ge)
                      ew16 = s2sb.tile([128, igw], F16, tag="ew16")
                      nc.vector.tensor_mul(ew16[:], e16[:], mk16[:])
                      nc.tensor.matmul(pfused[:], oh_sb[:, jt, :], ew16[:],
                                       start=(jt == 0), stop=(jt == n_js - 1))
                  fsb = s2sb.tile([C, igw], F32, tag="fsb")
                  nc.scalar.activation(fsb[:], pfused[:], AF.Copy)
                  for t in range(igw // 128):
                      it = ig * (igw // 128) + t
                      pft = ftps.tile([128, C], F32, tag="pft")
                      nc.tensor.transpose(pft[:, :C], fsb[:, t * 128:(t + 1) * 128],
                                          ident[:C, :C])
                      logsoftmax_from_psum(out_acc[:, it, :], pft[:, :C], s2sb,
                                           add_into=out_acc[:, it, :])
        nc.sync.dma_start(out_d.ap().rearrange("(it p) c -> p it c", p=128),
                          out_acc[:])

    nc.compile()
    return nc


def prep_inputs(inputs, n=N, n_layers=N_LAYERS):
    """Host-side sharding/layout prep. Returns in_maps (one dict per core)."""
    blk = n // N_CORES
    igw = min(512, blk)
    n_ig = blk // igw
    n_js = n // 128
    n_bkt = n_js * n_ig
    x = np.asarray(inputs["x"], np.float32)
    y = np.asarray(inputs["y"]).astype(np.int64)
    ei = np.asarray(inputs["edge_index"]).astype(np.int64)
    ew = np.asarray(inputs["edge_weight"], np.float32)
    src, dst = ei[0], ei[1]

    y32 = np.ascontiguousarray(
        y.reshape(n_js, 128).T.astype(np.float16))  # [128, n_js]
    w_in16 = np.ascontiguousarray(
        np.asarray(inputs["W_in"], np.float32)
        .reshape(D_IN // 128, 128, H).transpose(1, 0, 2)).astype(np.float16)
    b_in16 = np.asarray(inputs["b_in"], np.float32).reshape(1, H).astype(np.float16)
    w_out = np.ascontiguousarray(
        np.asarray(inputs["W_out"], np.float32)
        .reshape(2, 128, C).transpose(1, 0, 2))
    b_out = np.asarray(inputs["b_out"], np.float32).reshape(1, C)
    cw116 = np.ascontiguousarray(
        np.asarray(inputs["conv_w1"], np.float32)
        .reshape(n_layers, 2, 128, H).transpose(0, 2, 1, 3)).astype(np.float16)
    cw216 = np.ascontiguousarray(
        np.asarray(inputs["conv_w2"], np.float32)
        .reshape(n_layers, 2, 128, H).transpose(0, 2, 1, 3)).astype(np.float16)

    n_aw = blk // AW
    n_bkt = n_js * n_aw
    core_of = dst // blk
    in_maps = []
    for c in range(N_CORES):
        sel = core_of == c
        s, d, w = src[sel], dst[sel] - c * blk, ew[sel]
        js, sl_ = s >> 7, s & 127
        aw_, dw = d // AW, d % AW
        bkt = js * n_aw + aw_
        order = np.argsort(bkt, kind="stable")
        bs = bkt[order]
        counts = np.bincount(bs, minlength=n_bkt)
        if counts.max() > CAP:
            raise RuntimeError(f"edge bucket overflow: {counts.max()} > {CAP}")
        starts = np.zeros(n_bkt + 1, np.int64)
        np.cumsum(counts, out=starts[1:])
        pos = np.arange(len(bs)) - starts[bs]
        esrc_a = np.full((n_bkt, CAP), -1.0, np.float16)
        edst_a = np.full((n_bkt, CAP), -1.0, np.float16)
        ew_a = np.zeros((n_bkt, CAP), np.float16)
        esrc_a[bs, pos] = sl_[order]
        edst_a[bs, pos] = dw[order]
        ew_a[bs, pos] = w[order]
        # [n_js, n_aw, CAP=128] -> [128, n_js, n_aw]
        def to_dev(a):
            return np.ascontiguousarray(
                a.reshape(n_js, n_aw, 128).transpose(2, 0, 1))
        lo, hi = c * blk, (c + 1) * blk
        xT16 = np.ascontiguousarray(
            x[lo:hi].T.reshape(D_IN // 128, 128, blk)
            .transpose(1, 0, 2)).astype(np.float16)
        in_maps.append({
            "xT16": xT16, "esrc": to_dev(esrc_a), "edst": to_dev(edst_a),
            "ew": to_dev(ew_a), "y_r": y32, "w_in16": w_in16, "b_in16": b_in16,
            "cw116": cw116, "cw216": cw216, "w_out_r": w_out, "b_out_r": b_out,
        })
    return in_maps


def make_device_runner(nc, n_cores=N_CORES, chain=1):
    """Persistent PJRT executor mirroring bass2jax.run_bass_via_pjrt.

    Returns (upload, put_zeros, run, out_names): upload() transfers fresh
    host inputs to the 8 cores, put_zeros() stages the donated output
    buffers, run() executes the NEFF and blocks until complete.

    chain > 1 executes the NEFF `chain` times inside one dispatch, feeding
    each run's outputs in as the next run's output operands (bass_exec's
    ordered effect + the data dependency force strict sequencing) — used to
    amortize the host<->device dispatch RTT out of timing measurements.
    """
    import jax
    from jax.experimental.shard_map import shard_map
    from jax.sharding import Mesh, NamedSharding, PartitionSpec
    from concourse.bass2jax import (_bass_exec_p, install_neuronx_cc_hook,
                                    partition_id_tensor)

    install_neuronx_cc_hook()
    partition_name = nc.partition_id_tensor.name if nc.partition_id_tensor else None

    in_names, out_names, out_avals, zero_outs = [], [], [], []
    for alloc in nc.m.functions[0].allocations:
        if not isinstance(alloc, mybir.MemoryLocationSet):
            continue
        name = alloc.memorylocations[0].name
        if alloc.kind == "ExternalInput":
            if name != partition_name:
                in_names.append(name)
        elif alloc.kind == "ExternalOutput":
            shape = tuple(alloc.tensor_shape)
            dtype = mybir.dt.np(alloc.dtype)
            out_names.append(name)
            out_avals.append(jax.core.ShapedArray(shape, dtype))
            zero_outs.append(np.zeros(shape, dtype))
    n_params = len(in_names)
    n_outs = len(out_avals)
    all_in_names = list(in_names) + out_names
    if partition_name is not None:
        all_in_names.append(partition_name)
    donate = tuple(range(n_params, n_params + n_outs))

    def _body(*args):
        ins = list(args[:n_params])
        zs = list(args[n_params:])
        for _ in range(chain):
            operands = ins + zs
            if partition_name is not None:
                operands.append(partition_id_tensor())
            zs = list(_bass_exec_p.bind(
                *operands,
                out_avals=tuple(out_avals),
                in_names=tuple(all_in_names),
                out_names=tuple(out_names),
                lowering_input_output_aliases=(),
                sim_require_finite=True,
                sim_require_nnan=True,
                nc=nc,
            ))
        return tuple(zs)

    devices = jax.devices()[:n_cores]
    mesh = Mesh(np.asarray(devices), ("core",))
    in_specs = (PartitionSpec("core"),) * (n_params + n_outs)
    out_specs = (PartitionSpec("core"),) * n_outs
    sharded = jax.jit(
        shard_map(_body, mesh=mesh, in_specs=in_specs, out_specs=out_specs,
                  check_rep=False),
        donate_argnums=donate, keep_unused=True)
    sh = NamedSharding(mesh, PartitionSpec("core"))
    zero_shapes = [(n_cores * z.shape[0], *z.shape[1:]) for z in zero_outs]
    zero_dtypes = [z.dtype for z in zero_outs]

    def upload(in_maps):
        arrs = [
            jax.device_put(
                np.concatenate(
                    [np.asarray(in_maps[c][name]) for c in range(n_cores)],
                    axis=0), sh)
            for name in in_names
        ]
        jax.block_until_ready(arrs)
        return arrs

    def put_zeros():
        zs = [jax.device_put(np.zeros(s, d), sh)
              for s, d in zip(zero_shapes, zero_dtypes)]
        jax.block_until_ready(zs)
        return zs

    def run(dev_in, zs, block=True):
        outs = sharded(*dev_in, *zs)
        if block:
            jax.block_until_ready(outs)
        return outs

    return upload, put_zeros, run, out_names


_CACHED_NC = None
_CACHED_RUNNER = None


def kernel(**inputs):
    global _CACHED_NC, _CACHED_RUNNER
    if _CACHED_NC is None:
        _CACHED_NC = build_program()
        _CACHED_RUNNER = make_device_runner(_CACHED_NC)
    upload, put_zeros, run, out_names = _CACHED_RUNNER
    in_maps = prep_inputs(inputs)
    dev_in = upload(in_maps)
    zs = put_zeros()
    outs = run(dev_in, zs)
    oi = out_names.index("out")
    blk = N // N_CORES
    out = np.asarray(outs[oi]).reshape(N, C)
    return out.astype(np.float32)


if __name__ == "__main__":
    nc = build_program()
    print("built + compiled OK")

